# revision 32
# baseline (speedup 1.0000x reference)
"""Trainium2 Bass kernel for nn_DistributionLoss_6940667150680 (segment_reduce).

Math: with per-class sums S_c = sum_{i: Y_i=c} w_i and counts n_c,
    L2 = sum_i ||w_i - S_{Y_i}/n_{Y_i}||^2 = sum_i ||w_i||^2 - sum_c ||S_c||^2/n_c
so a single streaming pass over w1 (512 MB) suffices. Data-parallel over 8
NeuronCores (125k rows each). Per core:
  - PE: segment sums S[d, c] += w_tile^T @ onehot(Y_tile), fp16 inputs,
    fp32 PSUM accumulation ([128, 500] x 2 banks; split into an A/B pair so
    A's evacuation overlaps the final tiles' matmuls).
  - DVE: builds each [128, 1000] fp16 one-hot tile with a single
    tensor_scalar(is_equal) against a per-partition Y scalar (4x mode).
  - ACT: casts w f32->fp16 and accumulates sum(w^2) via Square+accum_out.
The tiny cross-core combine (sum of 8 [128,1000] partials, bincount of Y,
final scalar) happens on host in float64.
"""

import numpy as np
from contextlib import ExitStack

import concourse.bass as bass
import concourse.tile as tile
from concourse import mybir
from concourse.bass_utils import run_bass_kernel_spmd

N_CORES = 8
D = 128
C = 1000
P = 128          # partitions / rows per tile
SUPER = 8        # row-tiles per super-tile (one DMA + one cast + one square)


def build_program(supers: int):
    """Build the per-core Bass program. Rows processed = supers*8*128 + 128
    (the final "tail" tile comes from a separate, host-padded input)."""
    f32, f16 = mybir.dt.float32, mybir.dt.float16
    tiles = supers * SUPER + 1
    main_rows = supers * SUPER * P

    nc = bass.Bass()
    w_in = nc.dram_tensor("w", [main_rows, D], f32, kind="ExternalInput")
    wtail_in = nc.dram_tensor("wtail", [P, D], f32, kind="ExternalInput")
    yT_in = nc.dram_tensor("yT", [P, tiles], f32, kind="ExternalInput")
    iota_in = nc.dram_tensor("iota", [P, C], f16, kind="ExternalInput")
    s_out = nc.dram_tensor("s_out", [D, C], f32, kind="ExternalOutput")
    s_outA = nc.dram_tensor("s_outA", [D, C], f32, kind="ExternalOutput")
    sq_out = nc.dram_tensor("sq_out", [P, 1], f32, kind="ExternalOutput")

    # [s][p][g][d]: row index = (s*8 + g)*128 + p
    w_view = w_in.rearrange("(s g p) d -> s p g d", p=P, g=SUPER)

    WBUF = 3   # ring depth for w f32 / fp16 tiles (per super-tile)
    OBUF = 8   # ring depth for one-hot tiles (per row tile)

    def dep(frm, to, why):
        tile.add_dep_helper(
            getattr(frm, "ins", frm), getattr(to, "ins", to), reason=why
        )

    def demote(inst, dep_insts):
        """Move provably-redundant sync deps to nosync (ordering only).
        Used for same-engine WAW/WAR (in-order engines) and for deps that
        are transitively covered by another emitted wait; the TS/AC/DMA ISA
        structs only hold one sync wait each."""
        inst = getattr(inst, "ins", inst)
        drop = {getattr(d, "ins", d).name for d in dep_insts}
        syncs = inst.take_sync_dependencies()
        nosyncs = inst.take_nosync_dependencies()
        for name in drop & set(syncs):
            syncs.discard(name)
            nosyncs.add(name)
        inst.set_sync_dependencies(syncs)
        inst.set_nosync_dependencies(nosyncs)

    with tile.TileContext(nc) as tc, ExitStack() as ctx:
        const = ctx.enter_context(tc.tile_pool(name="const", bufs=1))
        psum = ctx.enter_context(tc.tile_pool(name="psum", bufs=1, space="PSUM"))

        iota_sb = const.tile([P, C], f16)
        nc.sync.dma_start(out=iota_sb, in_=iota_in[:, :])
        yT_sb = const.tile([P, tiles], f32)
        yhead = min(128, tiles)
        nc.sync.dma_start(out=yT_sb[:, 0:yhead], in_=yT_in[:, 0:yhead])
        sq_cols = const.tile([P, supers + 2], f32)

        out_sb = const.tile([D, C], f32, name="out_sb")
        out_sbA = const.tile([D, C], f32, name="out_sbA")

        def evac_a():
            ca1 = nc.vector.tensor_copy(out_sbA[:, 0:500], psum_loA)
            ca2 = nc.vector.tensor_copy(out_sbA[:, 500:C], psum_hiA)
            spnA = nc.sync.nop(nofuse=True, hint="spoutA")
            dep(spnA, ca1, "A ready")
            dep(spnA, ca2, "A ready")
            dmaA = nc.sync.dma_start(out=s_outA[:, :], in_=out_sbA)
            dep(dmaA, spnA, "after WAR nop")
            demote(dmaA, [spnA, ca1, ca2] + list(dmas.values()))
        sq_red_sb = const.tile([P, 1], f32, name="sq_red_sb")
        touch = const.tile([1, 8], f32, name="touch")
        # Touch the const tiles on DVE once so no compute op ever needs more
        # than one DMA wait.
        t0 = nc.vector.tensor_copy(touch[0:1, 0:2], iota_sb[0:1, 0:2])
        t1 = nc.vector.tensor_copy(touch[0:1, 2:4], yT_sb[0:1, 0:2])
        touch_writers = [t0, t1]

        # Explicit ring buffers (manual multi-buffering over subtile deps).
        w_ring = const.tile([P, WBUF, SUPER, D], f32, name="w_ring")
        w16_ring = const.tile([P, WBUF, SUPER, D], f16, name="w16_ring")
        oh_ring = const.tile([P, OBUF, C], f16, name="oh_ring")
        sqd_ring = const.tile([P, 2, SUPER * D], f16, name="sqd_ring")

        psum_loA = psum.tile([D, 500], f32)
        psum_hiA = psum.tile([D, 500], f32)
        psum_loB = psum.tile([D, 500], f32)
        psum_hiB = psum.tile([D, 500], f32)
        TSPLIT = max(0, (supers - 10) * SUPER)

        mms = {}      # t -> (mm_lo, mm_hi)
        tss = {}      # t -> tensor_scalar inst
        casts = {}    # s -> ACT cast inst
        claims = {}   # s -> ACT claim inst
        squares = {}  # s -> ACT square inst
        dmas = {}     # s -> w-load DMA inst

        def do_row_tile(w16_slice, cast, t, start, stop):
            oh = oh_ring[:, t % OBUF, :]
            ts = nc.vector.tensor_scalar(
                oh,
                iota_sb,
                yT_sb[:, t : t + 1],
                None,
                mybir.AluOpType.is_equal,
            )
            if t >= OBUF:
                # WAW vs tss[t-OBUF] is transitively covered by this op's own
                # PE wait (the t-OBUF matmuls waited on that tensor_scalar).
                demote(ts, [tss[t - OBUF]])
            tss[t] = ts
            if t < TSPLIT:
                p_lo, p_hi = psum_loA, psum_hiA
                start = start or t == 0
                stop = t == TSPLIT - 1
            else:
                p_lo, p_hi = psum_loB, psum_hiB
                start = t == TSPLIT
                stop = stop
            mm_lo = nc.tensor.matmul(
                p_lo, lhsT=w16_slice, rhs=oh[:, 0:500], start=start, stop=stop
            )
            mm_hi = nc.tensor.matmul(
                p_hi, lhsT=w16_slice, rhs=oh[:, 500:C], start=start, stop=stop
            )
            demote(mm_lo, [cast])
            demote(mm_hi, [cast])
            mms[t] = (mm_lo, mm_hi)
            if t == TSPLIT - 1:
                evac_a()

        def do_super(s, w_src, n_tiles, tile_base, start_first, stop_last):
            j = s % WBUF
            wt = w_ring[:, j, 0:n_tiles, :]
            spnop = None
            if s >= WBUF:
                # DMA instructions hold a single sync wait, which the HWDGE
                # lane-reuse protocol needs; carry the WAR wait on an SP nop
                # instead (the SP queue is FIFO, so the nop's hardware wait
                # also protects the DMA behind it).
                spnop = nc.sync.nop(nofuse=True, hint=f"spc{s}")
                dep(spnop, casts[s - WBUF], "w ring reader done")
            dma = nc.sync.dma_start(out=wt, in_=w_src)
            # DMA-vs-DMA WAW is ordered by the HWDGE ring (FIFO per issuing
            # engine); no semaphore needed.
            demote(dma, list(dmas.values()))
            if spnop is not None:
                dep(dma, spnop, "after WAR nop")
                demote(dma, [spnop] + list(casts.values()) + list(claims.values())
                       + list(squares.values()))
            dmas[s] = dma
            w16 = w16_ring[:, j, 0:n_tiles, :]
            wt_flat = wt.rearrange("p g d -> p (g d)")
            w16_flat = w16.rearrange("p g d -> p (g d)")
            if s >= WBUF:
                # tiny ACT claim write into the fp16 slot carries the PE WAR
                # wait (matmuls of s-WBUF still reading it); the big cast
                # behind it then needs only its DMA wait.
                claim = nc.scalar.activation(
                    w16_ring[0:1, j, 0, 0:2],
                    touch[0:1, 0:2],
                    mybir.ActivationFunctionType.Copy,
                )
                demote(claim, list(casts.values()) + list(claims.values())
                       + list(squares.values()) + list(tss.values()) + touch_writers)
                claims[s] = claim
            cast = nc.scalar.activation(
                w16_flat, wt_flat, mybir.ActivationFunctionType.Copy
            )
            if s >= WBUF:
                demote(cast, [m for pr in mms.values() for m in pr])
                demote(cast, list(casts.values()) + list(claims.values())
                       + list(squares.values()))
                demote(cast, touch_writers)
            casts[s] = cast
            # tiny DVE read of the casted tile: the is_equal/matmul chain then
            # transitively sees the cast without a second wait on the matmul
            tch = nc.vector.tensor_copy(touch[0:1, 6:8], w16_ring[0:1, j, 0, 0:2])
            demote(tch, touch_writers)
            touch_writers.append(tch)
            sqd = sqd_ring[:, s % 2, 0 : n_tiles * D]
            sq = nc.scalar.activation(
                sqd,
                w16_flat,
                mybir.ActivationFunctionType.Square,
                accum_out=sq_cols[:, s : s + 1],
            )
            # same-engine RAW on cast / WAW on sqd: in-order engine, no sem
            demote(sq, [cast] + list(casts.values()) + list(claims.values())
                   + list(squares.values()))
            squares[s] = sq
            for g in range(n_tiles):
                t = tile_base + g
                do_row_tile(
                    w16_ring[:, j, g, :],
                    cast,
                    t,
                    start=(start_first and g == 0),
                    stop=(stop_last and g == n_tiles - 1),
                )

        w0 = w_view[0]  # [p, g, d]
        do_super(0, w0[:, 0:1, :], 1, 0, True, False)
        do_super(1, w0[:, 1:SUPER, :], SUPER - 1, 1, False, False)
        if yhead < tiles:
            # rest of the yT scalars; a DVE touch re-covers the DMA wait so
            # later is_equal ops still carry only their PE wait
            dma_y2 = nc.sync.dma_start(out=yT_sb[:, yhead:tiles], in_=yT_in[:, yhead:tiles])
            demote(dma_y2, list(dmas.values()))
            t1b = nc.vector.tensor_copy(touch[0:1, 2:4], yT_sb[0:1, yhead : yhead + 2])
            demote(t1b, touch_writers)
            touch_writers.append(t1b)
        for s in range(1, supers):
            do_super(s + 1, w_view[s], SUPER, s * SUPER, False, False)
        # tail tile (host-padded to 128 rows)
        do_super(supers + 1, wtail_in[:, :].rearrange("(g p) d -> p g d", g=1),
                 1, supers * SUPER, False, True)

        # evacuate PSUM B -> SBUF -> DRAM (A was evacuated mid-stream)
        cp1 = nc.vector.tensor_copy(out_sb[:, 0:500], psum_loB)
        cp2 = nc.vector.tensor_copy(out_sb[:, 500:C], psum_hiB)
        spn1 = nc.sync.nop(nofuse=True, hint="spout1")
        dep(spn1, cp1, "s_sb ready")
        dep(spn1, cp2, "s_sb ready")
        out_dma1 = nc.sync.dma_start(out=s_out[:, :], in_=out_sb)
        dep(out_dma1, spn1, "after WAR nop")
        demote(out_dma1, [spn1, cp1, cp2])

        red = nc.vector.reduce_sum(sq_red_sb, sq_cols, axis=mybir.AxisListType.X)
        spn2 = nc.sync.nop(nofuse=True, hint="spout2")
        dep(spn2, red, "sq_red ready")
        out_dma2 = nc.sync.dma_start(out=sq_out[:, :], in_=sq_red_sb)
        dep(out_dma2, spn2, "after WAR nop")
        demote(out_dma2, [spn2, red])

        # Tail sync: cover every proc with single-wait SP nops (the SP queue
        # is FIFO, so the stripped tail drain behind them is safe).
        for tail_dep, why in (
            (mms[tiles - 1][1], "PE done"),
            (squares[supers + 1], "ACT done"),
            (out_dma1, "s_out dma done"),
            (out_dma2, "sq_out dma done"),
        ):
            nop = nc.sync.nop(nofuse=True, hint="tailcover")
            dep(nop, tail_dep, why)

    # The kernel-tail drain waits on every proc; its NOP struct cannot hold
    # that many sync waits and the SP-queue nops above already cover them.
    for blk in nc.m.functions[0].blocks:
        for inst in blk.instructions:
            if not isinstance(inst, mybir.InstDrain):
                continue
            si = inst.sync_info
            if si is None or len(si.on_wait) <= 2:
                continue
            inst.sync_info = mybir.SyncInfo(on_wait=[], on_update=list(si.on_update))

    return nc


def make_in_maps(w1: np.ndarray, Y: np.ndarray, supers: int):
    """Shard row-wise across 8 cores; per-core padded tail tile."""
    n = w1.shape[0]
    rows_per_core = n // N_CORES
    main_rows = supers * SUPER * P
    tail_real = rows_per_core - main_rows
    assert 0 < tail_real <= P, (rows_per_core, main_rows)
    tiles = supers * SUPER + 1

    iota = np.ascontiguousarray(
        np.broadcast_to(np.arange(C, dtype=np.float16), (P, C))
    )
    in_maps = []
    for k in range(N_CORES):
        a = k * rows_per_core
        w_main = w1[a : a + main_rows]  # contiguous view, no copy
        wtail = np.zeros((P, D), dtype=np.float32)
        wtail[:tail_real] = w1[a + main_rows : a + rows_per_core]
        ypad = np.zeros(tiles * P, dtype=np.float32)
        ypad[:rows_per_core] = Y[a : a + rows_per_core].astype(np.float32)
        yT = np.ascontiguousarray(ypad.reshape(tiles, P).T)
        in_maps.append({"w": w_main, "wtail": wtail, "yT": yT, "iota": iota})
    return in_maps


def combine(results, Y, n_total):
    """Host-side unshard: sum partial S/sumsq over cores, final scalar in f64."""
    s_total = np.zeros((D, C), dtype=np.float64)
    totsq = 0.0
    for r in results:
        s_total += r["s_out"].astype(np.float64)
        s_total += r["s_outA"].astype(np.float64)
        totsq += float(r["sq_out"].astype(np.float64).sum())
    counts = np.bincount(Y.astype(np.int64), minlength=C).astype(np.float64)
    corr = float(((s_total * s_total).sum(axis=0) / np.maximum(counts, 1.0)).sum())
    return np.float32((totsq - corr) / n_total)


def run_sharded(w1: np.ndarray, Y: np.ndarray, supers: int, trace: bool = False):
    nc = build_program(supers)
    in_maps = make_in_maps(w1, Y, supers)
    out = run_bass_kernel_spmd(nc, in_maps, list(range(N_CORES)), trace=trace)
    value = combine(out.results, Y, w1.shape[0])
    return value, out


def kernel(w1, Y, num_classes=None):
    w1 = np.ascontiguousarray(np.asarray(w1, dtype=np.float32))
    Y = np.asarray(Y)
    assert w1.shape == (1_000_000, 128) and int(np.asarray(num_classes)) == C
    # 125000 rows/core = 122 super-tiles (124928 rows) + 72-row tail tile
    value, _ = run_sharded(w1, Y, supers=122, trace=False)
    return value
